# revision 6
# baseline (speedup 1.0000x reference)
"""Trainium2 Bass kernel for retrieval_knn (65536 queries x 8192 codes, K=32, D=128).

Strategy (see spec sharding hint): data-parallel over queries across 8 cores.
Host side: adaptive KD tiling of queries into <=128-query tiles, each with a
certified candidate set (codes provably containing every query's exact 32-NN,
via Lipschitz bounds on the 32nd-NN distance from a sample lattice).
Device side, per tile:
  PE     : t_a = c_a - q_a per axis (K=2 matmul), W-transpose, W @ [codes|1]
  ACT    : squares of t_a (activation Square), PSUM->SBUF copies
  DVE    : d2 accumulate, w = 1/d2, 4x(max8 + match_replace) -> exact top-32,
           sparse W = w - replaced, normalize
  GPSIMD : clamp, W subtract (offload)
Exact same selection semantics as reference top_k (weights 1/d2, top-32).
"""
import sys
import os

sys.path.insert(0, "/opt/trn_rl_repo")

import numpy as np

K = 32
TILE = 128
NCORES = 8
D = 128
C_CAP = 832
CMAX = 960
MIN_LEAF = 16
SENT = 1000.0  # sentinel coordinate for padded candidates


# ----------------------------------------------------------------------------
# Host: adaptive tiling with certified candidate sets
# ----------------------------------------------------------------------------

def _adaptive_tiles(q, cpos):
    P = q.shape[0]
    gs = np.linspace(0, 1, 3)
    sx, sy, sz = np.meshgrid(gs, gs, gs, indexing="ij")
    lat = np.stack([sx, sy, sz], -1).reshape(-1, 3).astype(np.float32)

    def candidates(idx):
        pts = q[idx]
        lo, hi = pts.min(0), pts.max(0)
        samples = lo[None, :] + lat * (hi - lo)[None, :]
        d2s = ((samples[:, None, :] - cpos[None, :, :]) ** 2).sum(-1)
        d32s = np.sqrt(np.partition(d2s, K - 1, 1)[:, K - 1])
        dqs = np.sqrt(((pts[:, None, :] - samples[None, :, :]) ** 2).sum(-1))
        Rq = (d32s[None, :] + dqs).min(1)
        dbox = np.sqrt((np.maximum(np.maximum(lo[None] - cpos, cpos - hi[None]), 0) ** 2).sum(-1))
        pre = np.nonzero(dbox <= Rq.max())[0]
        if len(pre) > 6000 and len(idx) > MIN_LEAF:
            return None
        d2qc = ((pts[:, None, :] - cpos[pre][None, :, :]) ** 2).sum(-1)
        keep = (d2qc <= (Rq[:, None] ** 2)).any(0)
        cand = pre[keep]
        if len(cand) > C_CAP and len(idx) > MIN_LEAF:
            return None
        return cand

    def split(idx):
        pts = q[idx]
        ax = int(np.argmax(pts.max(0) - pts.min(0)))
        o = np.argsort(pts[:, ax], kind="stable")
        h = len(idx) // 2
        return idx[o[:h]], idx[o[h:]]

    nodes = [np.arange(P)]
    while len(nodes) < P // TILE:
        new = []
        for idx in nodes:
            new.extend(split(idx))
        nodes = new

    tiles = []
    stack = nodes
    while stack:
        idx = stack.pop()
        cand = candidates(idx)
        if cand is None:
            a, b = split(idx)
            stack += [a, b]
            continue
        assert len(cand) <= CMAX, f"candidate overflow: {len(cand)}"
        tiles.append((idx, cand))
    return tiles


# ----------------------------------------------------------------------------
# Device kernel build
# ----------------------------------------------------------------------------

def _build_nc(slot_C, lens):
    """slot_C: per-slot padded candidate count; lens: flat-array total sizes."""
    import concourse.bass as bass
    import concourse.mybir as mybir
    import concourse.tile as tile_mod
    from concourse.tile import TileContext
    from concourse.vector_clock import ScopedClock
    from concourse.masks import make_identity

    def _split_drain_and_barrier(self, tick_clock, wait_clock):
        nc = self.nc
        carriers = [nc.sync.nop(nofuse=True) for _ in range(40)]
        drain_inst = nc.sync.drain()
        wait_clock.add_sem_waits(drain_inst.ins, ScopedClock({None: tick_clock.global_clock}))
        si = drain_inst.ins.sync_info
        waits = list(si.on_wait or [])
        if len(waits) > 1:
            extra = waits[:-1]
            si.on_wait = waits[-1:]
            for i, w in enumerate(extra):
                c = carriers[i]
                csi = c.ins.sync_info
                if csi is None:
                    c.ins.sync_info = mybir.SyncInfo(on_wait=[w], on_update=[])
                else:
                    csi.on_wait = (csi.on_wait or []) + [w]
        nc.all_engine_barrier()
        popped = nc._tile_sem_poison_stack.pop()
        assert popped is self._sem_poison
        nc.clear_and_free_semaphores(list(self.sems.allocated().values()))
        nc.all_engine_barrier()

    tile_mod.TileContext._drain_and_barrier = _split_drain_and_barrier

    nslots = len(slot_C)
    nc = bass.Bass(trn_type="TRN2")
    f32 = mybir.dt.float32
    lhsq_d = nc.dram_tensor("lhsq", [nslots, 3, 2, TILE], f32, kind="ExternalInput")
    pos_d = nc.dram_tensor("pos", [lens["pos"]], f32, kind="ExternalInput")
    cod_d = nc.dram_tensor("cod", [lens["cod"]], f32, kind="ExternalInput")
    out_d = nc.dram_tensor("out", [nslots * TILE, D], f32, kind="ExternalOutput")

    pos_off = np.concatenate([[0], np.cumsum(6 * slot_C)])
    cod_off = np.concatenate([[0], np.cumsum(slot_C * (D + 1))])

    with TileContext(nc) as tc:
        with (
            tc.tile_pool(name="con", bufs=1) as con,
            tc.tile_pool(name="io", bufs=3) as io,
            tc.tile_pool(name="wk", bufs=2) as wk,
            tc.tile_pool(name="pt", bufs=2, space="PSUM") as pt,
            tc.tile_pool(name="pw", bufs=1, space="PSUM") as pw,
            tc.tile_pool(name="po", bufs=2, space="PSUM") as po,
        ):
            ident = con.tile([128, 128], f32)
            make_identity(nc, ident)

            for s in range(nslots):
                C = int(slot_C[s])
                NCH = (C + 127) // 128
                lhs_s = io.tile([66, TILE], f32, tag="lhs")
                pos_s = io.tile([66, CMAX], f32, tag="pos")
                cod_s = io.tile([128, (CMAX + 127) // 128, D + 1], f32, tag="cod")
                posf = pos_d[pos_off[s]:pos_off[s + 1]].rearrange("(a b c) -> a b c", a=3, b=2)
                for a in range(3):
                    nc.sync.dma_start(out=lhs_s[32 * a:32 * a + 2, :], in_=lhsq_d[s, a])
                    nc.sync.dma_start(out=pos_s[32 * a:32 * a + 2, :C], in_=posf[a])
                codf = cod_d[cod_off[s]:cod_off[s + 1]].rearrange("(c d) -> c d", d=D + 1)
                for ch in range(NCH):
                    cw = min(128, C - ch * 128)
                    nc.sync.dma_start(out=cod_s[:cw, ch, :], in_=codf[ch * 128:ch * 128 + cw])

                # per-axis: t_a = c_a - q_a on PE; square on ACT/DVE; accumulate d2
                sq = wk.tile([128, 3, CMAX], f32, tag="sq")
                for a in range(3):
                    t_ps = pt.tile([128, CMAX], f32, tag="t")
                    # lhsT rows [ones; q_a]; rhs rows [c_a; -ones] -> t = c_a - q_a
                    for n0 in range(0, C, 512):
                        n1 = min(n0 + 512, C)
                        nc.tensor.matmul(
                            t_ps[:, n0:n1],
                            lhs_s[32 * a:32 * a + 2, :],
                            pos_s[32 * a:32 * a + 2, n0:n1],
                            start=True, stop=True)
                    nc.scalar.square(sq[:, a, :C], t_ps[:, :C])

                d2 = wk.tile([128, CMAX], f32, tag="d2")
                nc.vector.tensor_add(d2[:, :C], sq[:, 0, :C], sq[:, 1, :C])
                nc.vector.tensor_add(d2[:, :C], d2[:, :C], sq[:, 2, :C])
                nc.gpsimd.tensor_scalar_max(d2[:, :C], d2[:, :C], 1.0e-14)
                wfull = wk.tile([128, CMAX], f32, tag="wfull")
                nc.vector.reciprocal(out=wfull[:, :C], in_=d2[:, :C])

                maxb = wk.tile([128, 32], f32, tag="maxb")
                work = wk.tile([128, CMAX], f32, tag="work")
                nc.vector.max(out=maxb[:, 0:8], in_=wfull[:, :C])
                nc.vector.match_replace(out=work[:, :C], in_to_replace=maxb[:, 0:8],
                                        in_values=wfull[:, :C], imm_value=0.0)
                for r in range(1, 4):
                    nc.vector.max(out=maxb[:, 8 * r:8 * r + 8], in_=work[:, :C])
                    nc.vector.match_replace(out=work[:, :C], in_to_replace=maxb[:, 8 * r:8 * r + 8],
                                            in_values=work[:, :C], imm_value=0.0)
                # sparse weights: W = wfull - work (top-32 kept, rest 0)
                nc.gpsimd.tensor_sub(out=work[:, :C], in0=wfull[:, :C], in1=work[:, :C])

                wt_ps = pw.tile([128, (CMAX + 127) // 128, 128], f32, tag="wt")
                wt_s = wk.tile([128, (CMAX + 127) // 128, 128], f32, tag="wts")
                for ch in range(NCH):
                    cw = min(128, C - ch * 128)
                    nc.tensor.transpose(wt_ps[:cw, ch, :], work[:, ch * 128:ch * 128 + cw], ident)
                    nc.scalar.copy(out=wt_s[:cw, ch, :], in_=wt_ps[:cw, ch, :])

                o_ps = po.tile([128, D + 1], f32, tag="o")
                for ch in range(NCH):
                    cw = min(128, C - ch * 128)
                    nc.tensor.matmul(o_ps[:, :], wt_s[:cw, ch, :], cod_s[:cw, ch, :],
                                     start=(ch == 0), stop=(ch == NCH - 1))

                inv = wk.tile([128, 1], f32, tag="inv")
                out_s = wk.tile([128, D], f32, tag="outs")
                nc.vector.reciprocal(out=inv, in_=o_ps[:, D:D + 1])
                nc.vector.tensor_scalar(out=out_s, in0=o_ps[:, 0:D], scalar1=inv,
                                        scalar2=None, op0=mybir.AluOpType.mult)
                nc.sync.dma_start(out=out_d[s * TILE:(s + 1) * TILE, :], in_=out_s)

    # walrus here encodes at most ONE sem-wait per instruction: hoist extras
    n = 0
    for f in nc.m.functions:
        for b in f.blocks:
            out = []
            for inst in b.instructions:
                si = inst.sync_info
                waits = list(si.on_wait) if si and si.on_wait else []
                if len(waits) > 1:
                    extra, keep = waits[:-1], waits[-1:]
                    si.on_wait = keep
                    for w in extra:
                        nop = mybir.InstNoOp(name=f"I-wsplit-{n}", ins=[], outs=[])
                        n += 1
                        nop.engine = inst.engine
                        nop.sync_info = mybir.SyncInfo(on_wait=[w], on_update=[])
                        out.append(nop)
                out.append(inst)
            b.instructions = out
    return nc


# ----------------------------------------------------------------------------
# Entry point
# ----------------------------------------------------------------------------

def prepare(indices, query_points, codes_position, codes):
    b = int(np.asarray(indices).reshape(-1)[0])
    q = np.asarray(query_points, np.float32)[0]
    cpos = np.asarray(codes_position, np.float32)[b]
    cds = np.asarray(codes, np.float32)[b]
    P = q.shape[0]

    tiles = _adaptive_tiles(q, cpos)

    # balance across cores: global sort by padded C desc, snake-deal
    padC = np.array([max(64, ((len(c) + 63) // 64) * 64) for _, c in tiles])
    order = np.argsort(-padC, kind="stable")
    nslots = (len(tiles) + NCORES - 1) // NCORES
    assign = [[] for _ in range(NCORES)]
    for r, t in enumerate(order):
        blk, pos = divmod(r, NCORES)
        core = pos if blk % 2 == 0 else NCORES - 1 - pos
        assign[core].append(t)
    slot_C = np.zeros(nslots, np.int64)
    for core in range(NCORES):
        for j, t in enumerate(assign[core]):
            slot_C[j] = max(slot_C[j], padC[t])
    slot_C = np.maximum(slot_C, 64)

    lens = {"pos": int((6 * slot_C).sum()), "cod": int((slot_C * (D + 1)).sum())}
    pos_off = np.concatenate([[0], np.cumsum(6 * slot_C)])
    cod_off = np.concatenate([[0], np.cumsum(slot_C * (D + 1))])

    in_maps = []
    meta = []  # per core: list of (slot, query_idx)
    for core in range(NCORES):
        lhsq = np.zeros((nslots, 3, 2, TILE), np.float32)
        lhsq[:, :, 0, :] = 1.0
        lhsq[:, :, 1, :] = 0.5
        pos = np.empty(lens["pos"], np.float32)
        cod = np.zeros(lens["cod"], np.float32)
        core_meta = []
        for j in range(nslots):
            Cj = int(slot_C[j])
            pj = np.empty((3, 2, Cj), np.float32)
            pj[:, 0, :] = SENT
            pj[:, 1, :] = -1.0
            cj = np.zeros((Cj, D + 1), np.float32)
            if j < len(assign[core]):
                t = assign[core][j]
                qidx, cidx = tiles[t]
                nq, ncd = len(qidx), len(cidx)
                lhsq[j, :, 1, :nq] = q[qidx].T
                if nq < TILE:
                    ctr = q[qidx].mean(0)
                    lhsq[j, :, 1, nq:] = ctr[:, None]
                pj[:, 0, :ncd] = cpos[cidx].T
                cj[:ncd, :D] = cds[cidx]
                cj[:ncd, D] = 1.0
                core_meta.append((j, qidx))
            pos[pos_off[j]:pos_off[j + 1]] = pj.reshape(-1)
            cod[cod_off[j]:cod_off[j + 1]] = cj.reshape(-1)
        in_maps.append({"lhsq": lhsq, "pos": pos, "cod": cod})
        meta.append(core_meta)

    nc = _build_nc(slot_C, lens)
    return {"nc": nc, "in_maps": in_maps, "meta": meta, "P": P, "slot_C": slot_C}


def assemble(prep, results):
    out = np.zeros((prep["P"], D), np.float32)
    for core in range(NCORES):
        o = results[core]["out"]
        for j, qidx in prep["meta"][core]:
            out[qidx] = o[j * TILE:j * TILE + len(qidx)]
    return out


def kernel(indices, query_points, codes_position, codes):
    from concourse.bass_utils import run_bass_kernel_spmd

    prep = prepare(indices, query_points, codes_position, codes)
    res = run_bass_kernel_spmd(prep["nc"], prep["in_maps"], core_ids=list(range(NCORES)))
    return assemble(prep, res.results)


# revision 8
# speedup vs baseline: 1.2805x; 1.2805x over previous
"""Trainium2 Bass kernel for retrieval_knn (65536 queries x 8192 codes, K=32, D=128).

Strategy (see spec sharding hint): data-parallel over queries across 8 cores.
Host side: adaptive KD tiling of queries into <=128-query tiles, each with a
certified candidate set (codes provably containing every query's exact 32-NN,
via Lipschitz bounds on the 32nd-NN distance from a sample lattice).
Device side, per tile:
  PE     : t_a = c_a - q_a per axis (K=2 matmul), W-transpose, W @ [codes|1]
  ACT    : squares of t_a (activation Square), PSUM->SBUF copies
  DVE    : d2 accumulate, w = 1/d2, 4x(max8 + match_replace) -> exact top-32,
           sparse W = w - replaced, normalize
  GPSIMD : clamp, W subtract (offload)
Exact same selection semantics as reference top_k (weights 1/d2, top-32).
"""
import sys
import os

sys.path.insert(0, "/opt/trn_rl_repo")

import numpy as np

K = 32
TILE = 128
NCORES = 8
D = 128
C_CAP = 832
CMAX = 960
MIN_LEAF = 16
SENT = 1000.0  # sentinel coordinate for padded candidates


# ----------------------------------------------------------------------------
# Host: adaptive tiling with certified candidate sets
# ----------------------------------------------------------------------------

def _adaptive_tiles(q, cpos):
    P = q.shape[0]
    gs = np.linspace(0, 1, 3)
    sx, sy, sz = np.meshgrid(gs, gs, gs, indexing="ij")
    lat = np.stack([sx, sy, sz], -1).reshape(-1, 3).astype(np.float32)

    def candidates(idx):
        pts = q[idx]
        lo, hi = pts.min(0), pts.max(0)
        samples = lo[None, :] + lat * (hi - lo)[None, :]
        d2s = ((samples[:, None, :] - cpos[None, :, :]) ** 2).sum(-1)
        d32s = np.sqrt(np.partition(d2s, K - 1, 1)[:, K - 1])
        dqs = np.sqrt(((pts[:, None, :] - samples[None, :, :]) ** 2).sum(-1))
        Rq = (d32s[None, :] + dqs).min(1)
        dbox = np.sqrt((np.maximum(np.maximum(lo[None] - cpos, cpos - hi[None]), 0) ** 2).sum(-1))
        pre = np.nonzero(dbox <= Rq.max())[0]
        if len(pre) > 6000 and len(idx) > MIN_LEAF:
            return None
        d2qc = ((pts[:, None, :] - cpos[pre][None, :, :]) ** 2).sum(-1)
        keep = (d2qc <= (Rq[:, None] ** 2)).any(0)
        cand = pre[keep]
        if len(cand) > C_CAP and len(idx) > MIN_LEAF:
            return None
        return cand

    def split(idx):
        pts = q[idx]
        ax = int(np.argmax(pts.max(0) - pts.min(0)))
        o = np.argsort(pts[:, ax], kind="stable")
        h = len(idx) // 2
        return idx[o[:h]], idx[o[h:]]

    nodes = [np.arange(P)]
    while len(nodes) < P // TILE:
        new = []
        for idx in nodes:
            new.extend(split(idx))
        nodes = new

    tiles = []
    stack = nodes
    while stack:
        idx = stack.pop()
        cand = candidates(idx)
        if cand is None:
            a, b = split(idx)
            stack += [a, b]
            continue
        assert len(cand) <= CMAX, f"candidate overflow: {len(cand)}"
        tiles.append((idx, cand))
    return tiles


# ----------------------------------------------------------------------------
# Device kernel build
# ----------------------------------------------------------------------------

def _build_nc(slot_C, lens):
    """slot_C: per-slot padded candidate count; lens: flat-array total sizes."""
    import concourse.bass as bass
    import concourse.mybir as mybir
    import concourse.tile as tile_mod
    from concourse.tile import TileContext
    from concourse.vector_clock import ScopedClock
    from concourse.masks import make_identity

    def _split_drain_and_barrier(self, tick_clock, wait_clock):
        nc = self.nc
        carriers = [nc.sync.nop(nofuse=True) for _ in range(40)]
        drain_inst = nc.sync.drain()
        wait_clock.add_sem_waits(drain_inst.ins, ScopedClock({None: tick_clock.global_clock}))
        si = drain_inst.ins.sync_info
        waits = list(si.on_wait or [])
        if len(waits) > 1:
            extra = waits[:-1]
            si.on_wait = waits[-1:]
            for i, w in enumerate(extra):
                c = carriers[i]
                csi = c.ins.sync_info
                if csi is None:
                    c.ins.sync_info = mybir.SyncInfo(on_wait=[w], on_update=[])
                else:
                    csi.on_wait = (csi.on_wait or []) + [w]
        nc.all_engine_barrier()
        popped = nc._tile_sem_poison_stack.pop()
        assert popped is self._sem_poison
        nc.clear_and_free_semaphores(list(self.sems.allocated().values()))
        nc.all_engine_barrier()

    tile_mod.TileContext._drain_and_barrier = _split_drain_and_barrier

    nslots = len(slot_C)
    nc = bass.Bass(trn_type="TRN2")
    f32 = mybir.dt.float32
    qneg_d = nc.dram_tensor("qneg", [nslots, TILE, 3], f32, kind="ExternalInput")
    pos_d = nc.dram_tensor("pos", [lens["pos"]], f32, kind="ExternalInput")
    cod_d = nc.dram_tensor("cod", [lens["cod"]], f32, kind="ExternalInput")
    out_d = nc.dram_tensor("out", [nslots * TILE, D], f32, kind="ExternalOutput")

    pos_off = np.concatenate([[0], np.cumsum(3 * slot_C)])
    cod_off = np.concatenate([[0], np.cumsum(slot_C * (D + 1))])

    with TileContext(nc) as tc:
        with (
            tc.tile_pool(name="con", bufs=1) as con,
            tc.tile_pool(name="io", bufs=3) as io,
            tc.tile_pool(name="wk", bufs=2) as wk,
            tc.tile_pool(name="pw", bufs=2, space="PSUM") as pw,
            tc.tile_pool(name="po", bufs=2, space="PSUM") as po,
        ):
            ident = con.tile([128, 128], f32)
            make_identity(nc, ident)

            for s in range(nslots):
                C = int(slot_C[s])
                NCH = (C + 127) // 128
                qn_s = io.tile([TILE, 3], f32, tag="qn")
                cb_s = io.tile([128, 3, CMAX], f32, tag="cb")
                cod_s = io.tile([128, (CMAX + 127) // 128, D + 1], f32, tag="cod")
                nc.sync.dma_start(out=qn_s, in_=qneg_d[s])
                for a in range(3):
                    seg = pos_d[pos_off[s] + a * C:pos_off[s] + (a + 1) * C]
                    bcast = bass.AP(tensor=seg.tensor, offset=seg.offset,
                                    ap=[[0, 128], [1, C]])
                    nc.sync.dma_start(out=cb_s[:, a, :C], in_=bcast)
                codf = cod_d[cod_off[s]:cod_off[s + 1]].rearrange("(c d) -> c d", d=D + 1)
                for ch in range(NCH):
                    cw = min(128, C - ch * 128)
                    nc.sync.dma_start(out=cod_s[:cw, ch, :], in_=codf[ch * 128:ch * 128 + cw])

                # sq_a = (c_a - q_a)^2 fused on ACT: Square(c_bcast + bias=-q_a)
                sq = wk.tile([128, 3, CMAX], f32, tag="sq")
                for a in range(3):
                    nc.scalar.activation(
                        sq[:, a, :C], cb_s[:, a, :C],
                        mybir.ActivationFunctionType.Square,
                        bias=qn_s[:, a:a + 1], scale=1.0)

                d2 = wk.tile([128, CMAX], f32, tag="d2")
                nc.gpsimd.tensor_add(d2[:, :C], sq[:, 0, :C], sq[:, 1, :C])
                nc.gpsimd.tensor_add(d2[:, :C], d2[:, :C], sq[:, 2, :C])
                nc.gpsimd.tensor_scalar_max(d2[:, :C], d2[:, :C], 1.0e-14)
                wfull = wk.tile([128, CMAX], f32, tag="wfull")
                nc.vector.reciprocal(out=wfull[:, :C], in_=d2[:, :C])

                maxb = wk.tile([128, 32], f32, tag="maxb")
                work = wk.tile([128, CMAX], f32, tag="work")
                nc.vector.max(out=maxb[:, 0:8], in_=wfull[:, :C])
                nc.vector.match_replace(out=work[:, :C], in_to_replace=maxb[:, 0:8],
                                        in_values=wfull[:, :C], imm_value=0.0)
                for r in range(1, 4):
                    nc.vector.max(out=maxb[:, 8 * r:8 * r + 8], in_=work[:, :C])
                    nc.vector.match_replace(out=work[:, :C], in_to_replace=maxb[:, 8 * r:8 * r + 8],
                                            in_values=work[:, :C], imm_value=0.0)
                # sparse weights: W = wfull - work (top-32 kept, rest 0)
                nc.gpsimd.tensor_sub(out=work[:, :C], in0=wfull[:, :C], in1=work[:, :C])

                wt_ps = pw.tile([128, (CMAX + 127) // 128, 128], f32, tag="wt")
                wt_s = wk.tile([128, (CMAX + 127) // 128, 128], f32, tag="wts")
                for ch in range(NCH):
                    cw = min(128, C - ch * 128)
                    nc.tensor.transpose(wt_ps[:cw, ch, :], work[:, ch * 128:ch * 128 + cw], ident)
                    nc.scalar.copy(out=wt_s[:cw, ch, :], in_=wt_ps[:cw, ch, :])

                o_ps = po.tile([128, D + 1], f32, tag="o")
                for ch in range(NCH):
                    cw = min(128, C - ch * 128)
                    nc.tensor.matmul(o_ps[:, :], wt_s[:cw, ch, :], cod_s[:cw, ch, :],
                                     start=(ch == 0), stop=(ch == NCH - 1))

                inv = wk.tile([128, 1], f32, tag="inv")
                out_s = wk.tile([128, D], f32, tag="outs")
                nc.vector.reciprocal(out=inv, in_=o_ps[:, D:D + 1])
                nc.vector.tensor_scalar(out=out_s, in0=o_ps[:, 0:D], scalar1=inv,
                                        scalar2=None, op0=mybir.AluOpType.mult)
                nc.sync.dma_start(out=out_d[s * TILE:(s + 1) * TILE, :], in_=out_s)

    # walrus here encodes at most ONE sem-wait per instruction: hoist extras
    n = 0
    for f in nc.m.functions:
        for b in f.blocks:
            out = []
            for inst in b.instructions:
                si = inst.sync_info
                waits = list(si.on_wait) if si and si.on_wait else []
                if len(waits) > 1:
                    extra, keep = waits[:-1], waits[-1:]
                    si.on_wait = keep
                    for w in extra:
                        nop = mybir.InstNoOp(name=f"I-wsplit-{n}", ins=[], outs=[])
                        n += 1
                        nop.engine = inst.engine
                        nop.sync_info = mybir.SyncInfo(on_wait=[w], on_update=[])
                        out.append(nop)
                out.append(inst)
            b.instructions = out
    return nc


# ----------------------------------------------------------------------------
# Entry point
# ----------------------------------------------------------------------------

def prepare(indices, query_points, codes_position, codes):
    b = int(np.asarray(indices).reshape(-1)[0])
    q = np.asarray(query_points, np.float32)[0]
    cpos = np.asarray(codes_position, np.float32)[b]
    cds = np.asarray(codes, np.float32)[b]
    P = q.shape[0]

    tiles = _adaptive_tiles(q, cpos)

    # balance across cores: global sort by padded C desc, snake-deal
    padC = np.array([max(64, ((len(c) + 63) // 64) * 64) for _, c in tiles])
    order = np.argsort(-padC, kind="stable")
    nslots = (len(tiles) + NCORES - 1) // NCORES
    assign = [[] for _ in range(NCORES)]
    for r, t in enumerate(order):
        blk, pos = divmod(r, NCORES)
        core = pos if blk % 2 == 0 else NCORES - 1 - pos
        assign[core].append(t)
    slot_C = np.zeros(nslots, np.int64)
    for core in range(NCORES):
        for j, t in enumerate(assign[core]):
            slot_C[j] = max(slot_C[j], padC[t])
    slot_C = np.maximum(slot_C, 64)

    lens = {"pos": int((3 * slot_C).sum()), "cod": int((slot_C * (D + 1)).sum())}
    pos_off = np.concatenate([[0], np.cumsum(3 * slot_C)])
    cod_off = np.concatenate([[0], np.cumsum(slot_C * (D + 1))])

    in_maps = []
    meta = []  # per core: list of (slot, query_idx)
    for core in range(NCORES):
        qneg = np.full((nslots, TILE, 3), -0.5, np.float32)
        pos = np.empty(lens["pos"], np.float32)
        cod = np.zeros(lens["cod"], np.float32)
        core_meta = []
        for j in range(nslots):
            Cj = int(slot_C[j])
            pj = np.full((3, Cj), SENT, np.float32)
            cj = np.zeros((Cj, D + 1), np.float32)
            if j < len(assign[core]):
                t = assign[core][j]
                qidx, cidx = tiles[t]
                nq, ncd = len(qidx), len(cidx)
                qneg[j, :nq, :] = -q[qidx]
                if nq < TILE:
                    qneg[j, nq:, :] = -q[qidx].mean(0)
                pj[:, :ncd] = cpos[cidx].T
                cj[:ncd, :D] = cds[cidx]
                cj[:ncd, D] = 1.0
                core_meta.append((j, qidx))
            pos[pos_off[j]:pos_off[j + 1]] = pj.reshape(-1)
            cod[cod_off[j]:cod_off[j + 1]] = cj.reshape(-1)
        in_maps.append({"qneg": qneg, "pos": pos, "cod": cod})
        meta.append(core_meta)

    nc = _build_nc(slot_C, lens)
    return {"nc": nc, "in_maps": in_maps, "meta": meta, "P": P, "slot_C": slot_C}


def assemble(prep, results):
    out = np.zeros((prep["P"], D), np.float32)
    for core in range(NCORES):
        o = results[core]["out"]
        for j, qidx in prep["meta"][core]:
            out[qidx] = o[j * TILE:j * TILE + len(qidx)]
    return out


def kernel(indices, query_points, codes_position, codes):
    from concourse.bass_utils import run_bass_kernel_spmd

    prep = prepare(indices, query_points, codes_position, codes)
    res = run_bass_kernel_spmd(prep["nc"], prep["in_maps"], core_ids=list(range(NCORES)))
    return assemble(prep, res.results)
